# revision 1
# baseline (speedup 1.0000x reference)
"""GridToStation Trainium2 kernel.

Strategy (sharding_hint: shard grid over nlon, route stations to owning shard):
  - Host: transpose grid (C,H,W) -> (H,W,C); shard over W into 8 shards of 180
    columns + 1 halo column (duplicated edge for the last shard). Each shard is
    a gather table of shape (721*181, 256) f32 whose rows are grid points.
  - Host: compute per-station bilinear corner indices + weights exactly as the
    reference does (f32 math), bucket stations by owning shard, pad each
    bucket to a common padded count.
  - Device (per core, SPMD over 8 cores):
      * indirect (gather) DMA pulls, for each station, two 2KB rows:
        [v00|v01] at (iy0, ix0..ix0+1) and [v10|v11] at (iy1, ix0..ix0+1).
      * bilinear combine fused into PE transpose: for each corner j,
        matmul(out_psum += v_j_chunk^T @ diag(c_j)) accumulates the weighted
        transpose x^T [C, stations] directly in PSUM.
      * MLP: h = gelu(x @ W1^T + b1); y = h @ W2^T + b2 computed in
        [C, stations] layout (bias is then per-partition, fed to ACT).
      * One big output write at the end: y^T stored as [256, n_pad] in DRAM.
  - Host: gather per-core outputs, inverse-permute to original station order.
"""

import os

import numpy as np

B, C, H, W, N = 1, 256, 721, 1440, 16384
NCORES = 8
COLS = W // NCORES  # 180 owned columns per shard
WP = COLS + 1  # +1 halo column
TROWS = H * WP  # gather-table rows per shard
R = 4  # station tiles (of 128) per gather group
PAIR_T = 2  # station tiles per MLP batch (N=256)

# matmul operand dtype knob: "f32" (exact) or "f32r" (fast, HW-reduced precision)
MM_DTYPE = os.environ.get("GRIDSTN_MM_DTYPE", "f32")

_PROG_CACHE = {}


def _f32(x):
    return np.float32(x)


def _host_route(station_coords):
    """Replicate the reference index math in f32, bucket stations by shard."""
    lat = np.asarray(station_coords[0, :, 0], dtype=np.float32)
    lon = np.asarray(station_coords[0, :, 1], dtype=np.float32)
    lat_n = lat / _f32(90.0)
    lon_n = lon / _f32(180.0)
    ix = np.clip((lon_n + _f32(1.0)) * _f32(0.5) * _f32(W - 1), _f32(0.0), _f32(W - 1))
    iy = np.clip((lat_n + _f32(1.0)) * _f32(0.5) * _f32(H - 1), _f32(0.0), _f32(H - 1))
    ix0f = np.floor(ix)
    iy0f = np.floor(iy)
    wx = (ix - ix0f).astype(np.float32)
    wy = (iy - iy0f).astype(np.float32)
    ix0 = ix0f.astype(np.int32)
    iy0 = iy0f.astype(np.int32)
    iy1 = np.minimum(iy0 + 1, H - 1)
    owner = ix0 // COLS  # ix0 <= 1439 -> owner <= 7
    x0l = ix0 - owner * COLS  # 0..179; +1 stays inside WP=181
    row0 = iy0 * WP + x0l
    row1 = iy1 * WP + x0l
    one = _f32(1.0)
    c00 = (one - wx) * (one - wy)
    c01 = wx * (one - wy)
    c10 = (one - wx) * wy
    c11 = wx * wy
    return owner, row0, row1, (c00, c01, c10, c11)


def _host_tables(grid_features):
    g = np.asarray(grid_features[0], dtype=np.float32)  # (C, H, W)
    gt = np.ascontiguousarray(np.transpose(g, (1, 2, 0)))  # (H, W, C)
    tables = []
    for c in range(NCORES):
        lo = c * COLS
        if c < NCORES - 1:
            tbl = gt[:, lo : lo + WP, :]
        else:
            tbl = np.concatenate([gt[:, lo:W, :], gt[:, W - 1 : W, :]], axis=1)
        tables.append(np.ascontiguousarray(tbl).reshape(TROWS, C))
    return tables


def _build_program(G):
    import concourse.bacc as bacc
    import concourse.bass as bass
    import concourse.mybir as mybir
    from concourse.tile import TileContext

    f32 = mybir.dt.float32
    i32 = mybir.dt.int32
    mm_dt = mybir.dt.float32r if MM_DTYPE == "f32r" else f32

    T = G * R  # station tiles total
    NP = T * 128  # padded stations per core

    nc = bacc.Bacc("TRN2", target_bir_lowering=False, debug=False)

    tbl = nc.dram_tensor("tbl", [TROWS, C], f32, kind="ExternalInput")
    idx = nc.dram_tensor("idx", [128, G * 2 * R], i32, kind="ExternalInput")
    cof = nc.dram_tensor("cof", [128, 4 * T], f32, kind="ExternalInput")
    w1 = nc.dram_tensor("w1t", [C, C], f32, kind="ExternalInput")
    w2 = nc.dram_tensor("w2t", [C, C], f32, kind="ExternalInput")
    bia = nc.dram_tensor("bia", [128, 4], f32, kind="ExternalInput")
    idn = nc.dram_tensor("idn", [128, 128], f32, kind="ExternalInput")
    out = nc.dram_tensor("out", [2, 128, NP], f32, kind="ExternalOutput")

    def mm(ap):
        return ap.bitcast(mm_dt) if mm_dt != f32 else ap

    with TileContext(nc) as tc:
        with (
            tc.tile_pool(name="const", bufs=1) as cpool,
            tc.tile_pool(name="gat", bufs=3) as gpool,
            tc.tile_pool(name="sm", bufs=4) as spool,
            tc.tile_pool(name="xs", bufs=4) as xpool,
            tc.tile_pool(name="hs", bufs=4) as hpool,
            tc.tile_pool(name="px", bufs=2, space="PSUM") as pxp,
            tc.tile_pool(name="ph", bufs=1, space="PSUM") as php,
            tc.tile_pool(name="py", bufs=1, space="PSUM") as pyp,
        ):
            idx_sb = cpool.tile([128, G * 2 * R], i32)
            nc.sync.dma_start(out=idx_sb[:], in_=idx[:])
            cof_sb = cpool.tile([128, 4 * T], f32)
            nc.sync.dma_start(out=cof_sb[:], in_=cof[:])
            w1_sb = cpool.tile([128, 2 * C], f32)
            nc.sync.dma_start(out=w1_sb[:, 0:C], in_=w1[0:128, :])
            nc.sync.dma_start(out=w1_sb[:, C : 2 * C], in_=w1[128:256, :])
            w2_sb = cpool.tile([128, 2 * C], f32)
            nc.sync.dma_start(out=w2_sb[:, 0:C], in_=w2[0:128, :])
            nc.sync.dma_start(out=w2_sb[:, C : 2 * C], in_=w2[128:256, :])
            bia_sb = cpool.tile([128, 4], f32)
            nc.sync.dma_start(out=bia_sb[:], in_=bia[:])
            idn_sb = cpool.tile([128, 128], f32)
            nc.sync.dma_start(out=idn_sb[:], in_=idn[:])
            out_sb = cpool.tile([128, 2 * NP], f32)

            for gi in range(G):
                gt_t = gpool.tile([128, 2 * R * 512], f32)
                # HW indirect DMA honors one dynamic offset per partition:
                # issue one gather per 512-elem block (rows idx, idx+1).
                for q in range(2 * R):
                    nc.gpsimd.indirect_dma_start(
                        out=gt_t[:, q * 512 : (q + 1) * 512],
                        out_offset=None,
                        in_=tbl[:],
                        in_offset=bass.IndirectOffsetOnAxis(
                            ap=idx_sb[:, gi * 2 * R + q : gi * 2 * R + q + 1], axis=0
                        ),
                    )
                for pr in range(R // PAIR_T):
                    pxs = [pxp.tile([128, 256], f32, name=f"px{i}") for i in range(2)]
                    for tt in range(PAIR_T):
                        r = pr * PAIR_T + tt
                        tg = gi * R + r
                        # bilinear combine on DVE in [station, C] layout:
                        # acc = v00*c00; acc = vj*cj + acc (fused) x3
                        sm = spool.tile([128, 256], f32, name=f"sm{tt}")
                        for j in range(4):
                            y = j >> 1
                            xo = (j & 1) * 256
                            base = (y * R + r) * 512 + xo
                            vj = gt_t[:, base : base + 256]
                            cj = cof_sb[:, j * T + tg : j * T + tg + 1]
                            if j == 0:
                                # seed on ScalarE (has slack): sm = vj * cj
                                nc.scalar.activation(
                                    out=sm[:],
                                    in_=vj,
                                    func=mybir.ActivationFunctionType.Copy,
                                    scale=cj,
                                )
                            else:
                                nc.vector.scalar_tensor_tensor(
                                    out=sm[:],
                                    in0=vj,
                                    scalar=cj,
                                    in1=sm[:],
                                    op0=mybir.AluOpType.mult,
                                    op1=mybir.AluOpType.add,
                                )
                        # plain permutation transpose to [C, station] in PSUM
                        for ch in range(2):
                            nc.tensor.transpose(
                                out=pxs[ch][:, tt * 128 : (tt + 1) * 128],
                                in_=sm[:, ch * 128 : (ch + 1) * 128],
                                identity=idn_sb[:],
                            )
                    xss = [xpool.tile([128, 256], f32, name=f"xs{i}") for i in range(2)]
                    for ch in range(2):
                        nc.scalar.activation(
                            out=xss[ch][:],
                            in_=pxs[ch][:],
                            func=mybir.ActivationFunctionType.Copy,
                        )
                    phs = [php.tile([128, 256], f32, name=f"ph{i}") for i in range(2)]
                    for m in range(2):
                        for k in range(2):
                            nc.tensor.matmul(
                                out=phs[m][:],
                                lhsT=mm(w1_sb[:, k * C + m * 128 : k * C + (m + 1) * 128]),
                                rhs=mm(xss[k][:]),
                                start=(k == 0),
                                stop=(k == 1),
                            )
                    hss = [hpool.tile([128, 256], f32, name=f"hs{i}") for i in range(2)]
                    for m in range(2):
                        nc.scalar.activation(
                            out=hss[m][:],
                            in_=phs[m][:],
                            func=mybir.ActivationFunctionType.Gelu,
                            bias=bia_sb[:, m : m + 1],
                            scale=1.0,
                        )
                    pys = [pyp.tile([128, 256], f32, name=f"py{i}") for i in range(2)]
                    for m in range(2):
                        for k in range(2):
                            nc.tensor.matmul(
                                out=pys[m][:],
                                lhsT=mm(w2_sb[:, k * C + m * 128 : k * C + (m + 1) * 128]),
                                rhs=mm(hss[k][:]),
                                start=(k == 0),
                                stop=(k == 1),
                            )
                    col = (gi * R + pr * PAIR_T) * 128
                    for m in range(2):
                        nc.vector.tensor_scalar_add(
                            out_sb[:, m * NP + col : m * NP + col + 256],
                            pys[m][:],
                            bia_sb[:, 2 + m : 3 + m],
                        )
            nc.sync.dma_start(out=out[0], in_=out_sb[:, 0:NP])
            nc.sync.dma_start(out=out[1], in_=out_sb[:, NP : 2 * NP])
    return nc, NP


def _make_in_maps(grid_features, station_coords, W1, b1, W2, b2):
    owner, row0, row1, cjs = _host_route(station_coords)
    tables = _host_tables(grid_features)

    sids_per_core = [np.nonzero(owner == c)[0] for c in range(NCORES)]
    max_n = max(len(s) for s in sids_per_core)
    G = max(1, -(-max_n // (R * 128)))  # ceil
    T = G * R
    NP = T * 128

    w1t = np.ascontiguousarray(np.asarray(W1, np.float32).T)
    w2t = np.ascontiguousarray(np.asarray(W2, np.float32).T)
    bia = np.zeros((128, 4), np.float32)
    bia[:, 0] = b1[0:128]
    bia[:, 1] = b1[128:256]
    bia[:, 2] = b2[0:128]
    bia[:, 3] = b2[128:256]
    idn = np.eye(128, dtype=np.float32)

    in_maps = []
    for c in range(NCORES):
        sids = sids_per_core[c]
        nl = len(sids)
        r0 = np.zeros(NP, np.int32)
        r1 = np.zeros(NP, np.int32)
        r0[:nl] = row0[sids]
        r1[:nl] = row1[sids]
        cj = np.zeros((4, NP), np.float32)
        for j in range(4):
            cj[j, :nl] = cjs[j][sids]
        r0m = r0.reshape(T, 128).T  # [128, T]
        r1m = r1.reshape(T, 128).T
        idx_arr = np.zeros((128, G * 2 * R), np.int32)
        for g in range(G):
            idx_arr[:, g * 2 * R : g * 2 * R + R] = r0m[:, g * R : (g + 1) * R]
            idx_arr[:, g * 2 * R + R : (g + 1) * 2 * R] = r1m[:, g * R : (g + 1) * R]
        cof_arr = np.ascontiguousarray(
            np.concatenate([cj[j].reshape(T, 128).T for j in range(4)], axis=1)
        )
        in_maps.append(
            {
                "tbl": tables[c],
                "idx": np.ascontiguousarray(idx_arr),
                "cof": cof_arr,
                "w1t": w1t,
                "w2t": w2t,
                "bia": bia,
                "idn": idn,
            }
        )
    return in_maps, sids_per_core, G, NP


LAST_RUN_INFO = {}


def _install_ntff_shim():
    """This container's antenv lacks axon_hooks; provide the NTFF profile
    hook via the same ctypes path trn_boot would have used."""
    import sys
    import types

    try:
        import antenv.axon_hooks  # noqa: F401

        return
    except ImportError:
        pass
    from trn_agent_boot.trn_boot import _ntff_profile_via_ctypes

    hook = _ntff_profile_via_ctypes("/opt/axon/libaxon_pjrt.so")
    mod = types.ModuleType("antenv.axon_hooks")
    mod.get_axon_ntff_profile_hook = lambda: hook
    mod.set_axon_ntff_profile_hook = lambda h: None
    sys.modules["antenv.axon_hooks"] = mod


def kernel(grid_features, station_coords, W1, b1, W2, b2):
    in_maps, sids_per_core, G, NP = _make_in_maps(
        grid_features, station_coords, W1, b1, W2, b2
    )

    key = (G, MM_DTYPE)
    if key not in _PROG_CACHE:
        _PROG_CACHE[key] = _build_program(G)
    nc, NP2 = _PROG_CACHE[key]
    assert NP2 == NP

    if os.environ.get("GRIDSTN_SIM"):
        outs = _run_sim(nc, in_maps)
    else:
        from concourse.bass_utils import run_bass_kernel_spmd

        trace = bool(os.environ.get("GRIDSTN_TRACE"))
        if trace:
            _install_ntff_shim()
        if not nc.is_finalized():
            nc.finalize()
        res = run_bass_kernel_spmd(
            nc, in_maps, list(range(NCORES)), trace=trace
        )
        LAST_RUN_INFO["exec_time_ns"] = res.exec_time_ns
        LAST_RUN_INFO["mean_exec_time_ns"] = res.mean_exec_time_ns
        LAST_RUN_INFO["profile_json"] = res.profile_json
        outs = [r["out"] for r in res.results]

    result = np.zeros((N, C), np.float32)
    for c in range(NCORES):
        sids = sids_per_core[c]
        y = outs[c].reshape(2 * 128, NP)
        result[sids] = y[:, : len(sids)].T
    return result.reshape(B, N, C)


def _run_sim(nc, in_maps):
    from concourse import bass_interp

    outs = []
    for c in range(NCORES):
        sim = bass_interp.MultiCoreSim(nc, 1)
        for name, arr in in_maps[c].items():
            sim.cores[0].tensor(name)[:] = arr
        sim.simulate()
        LAST_RUN_INFO["sim_time_ns"] = sim.cores[0].time
        outs.append(np.array(sim.cores[0].tensor("out")))
        if os.environ.get("GRIDSTN_SIM_ONE_CORE"):
            # replicate core 0's output for the rest (fast smoke mode)
            outs = outs + [outs[0]] * (NCORES - 1)
            break
    return outs



# revision 2
# speedup vs baseline: 1.0540x; 1.0540x over previous
"""GridToStation Trainium2 kernel, v2.

Pipeline (per core, SPMD x8):
  - Host: exact reference index math (f32). Stations sorted by ix0 and split
    into 8 equal chunks of 2048 -> perfect load balance. Within a core,
    stations sort by iy0 and cut into 6 fixed-size bands ([3,3,3,3,3,1]
    tiles of 128). Each band's grid rows are copied into a fixed-offset
    32768-row slab of the core's table (bf16, (H,W,C) layout, per-core
    column window of WTBL columns), so the per-band gather uses int16
    indices relative to a compile-time-constant slab base.
  - Host table rows are 4-corner blocks [v00|v01|v10|v11] (1024 bf16 =
    2KB per (iy0,ix0) cell), so each station needs ONE gather descriptor.
    Per band, ONE dma_gather (InstDMAGatherAnt) fetches all its stations,
    amortizing the SWDGE descriptor-gen launch on Pool.
  - Bilinear combine fused into the PE transpose: per tile, DVE builds 4
    diagonal matrices diag(c_j) (tensor_scalar_mul of the identity, 4x DVE
    mode), and 8 PE matmuls x^T[chunk] += v_j_chunk^T @ diag(c_j)
    accumulate the weighted transpose directly in PSUM (f32).
  - PSUM -> SBUF bf16 copies (DVE/ACT split), 2-layer MLP in bf16 on PE,
    Gelu(+bias) on ACT, y written back as bf16 (b1/b2==0 fast variants
    compiled adaptively); chunked output DMA; host upcasts/permutes.
"""

import os

import numpy as np

B, C, H, W, N = 1, 256, 721, 1440, 16384
NCORES = 8
NPC = N // NCORES  # 2048 stations per core
WTBL = 192  # per-core column window
SLABROWS = 32768  # table rows per band slab (int16-addressable)
BANDS_T = [1, 3, 3, 3, 3, 3]  # tiles per band (fixed); small first band starts the pipeline early
NBANDS = len(BANDS_T)
T = NPC // 128  # 16 tiles
TROWS = NBANDS * SLABROWS
NP = NPC
CB = 4 * C  # 4-corner block elems per table row
MAX_IDX = SLABROWS - 2

_PROG_CACHE = {}

LAST_RUN_INFO = {}


def _f32(x):
    return np.float32(x)


def _host_route(station_coords):
    lat = np.asarray(station_coords[0, :, 0], dtype=np.float32)
    lon = np.asarray(station_coords[0, :, 1], dtype=np.float32)
    lat_n = lat / _f32(90.0)
    lon_n = lon / _f32(180.0)
    ix = np.clip((lon_n + _f32(1.0)) * _f32(0.5) * _f32(W - 1), _f32(0.0), _f32(W - 1))
    iy = np.clip((lat_n + _f32(1.0)) * _f32(0.5) * _f32(H - 1), _f32(0.0), _f32(H - 1))
    ix0f = np.floor(ix)
    iy0f = np.floor(iy)
    wx = (ix - ix0f).astype(np.float32)
    wy = (iy - iy0f).astype(np.float32)
    ix0 = ix0f.astype(np.int32)
    iy0 = iy0f.astype(np.int32)
    iy1 = np.minimum(iy0 + 1, H - 1)
    one = _f32(1.0)
    cjs = (
        (one - wx) * (one - wy),
        wx * (one - wy),
        (one - wx) * wy,
        wx * wy,
    )
    return ix0, iy0, iy1, cjs


def _build_program(b1z=False, b2z=False):
    import concourse.bacc as bacc
    import concourse.bass as bass
    import concourse.mybir as mybir
    from concourse.tile import TileContext

    f32 = mybir.dt.float32
    bf16 = mybir.dt.bfloat16
    i16 = mybir.dt.int16
    AF = mybir.ActivationFunctionType
    ALU = mybir.AluOpType

    nc = bacc.Bacc("TRN2", target_bir_lowering=False, debug=False, num_swdge_queues=2)

    tbl = nc.dram_tensor("tbl", [TROWS, CB], bf16, kind="ExternalInput")
    # int16 idx, [128, sum over bands of kb*128/16] packed per band
    idx_cols = [kb * 128 // 16 for kb in BANDS_T]
    idx_off = np.cumsum([0] + idx_cols).tolist()
    idx = nc.dram_tensor("idx", [128, idx_off[-1]], i16, kind="ExternalInput")
    cof = nc.dram_tensor("cof", [128, 4 * T], f32, kind="ExternalInput")
    w1 = nc.dram_tensor("w1t", [C, C], bf16, kind="ExternalInput")
    w2 = nc.dram_tensor("w2t", [C, C], bf16, kind="ExternalInput")
    bia = nc.dram_tensor("bia", [128, 4], f32, kind="ExternalInput")
    idn = nc.dram_tensor("idn", [128, 128], bf16, kind="ExternalInput")
    out = nc.dram_tensor("out", [2, 128, NP], bf16, kind="ExternalOutput")

    KBMAX = max(BANDS_T)

    with TileContext(nc) as tc:
        with (
            tc.tile_pool(name="const", bufs=1) as cpool,
            tc.tile_pool(name="gat", bufs=6) as gpool,
            tc.tile_pool(name="dg", bufs=6) as dpool,
            tc.tile_pool(name="xs", bufs=4) as xpool,
            tc.tile_pool(name="hs", bufs=4) as hpool,
            tc.tile_pool(name="px", bufs=3, space="PSUM") as pxp,
            tc.tile_pool(name="ph", bufs=2, space="PSUM") as php,
            tc.tile_pool(name="py", bufs=2, space="PSUM") as pyp,
        ):
            idx_sb = cpool.tile([128, idx_off[-1]], i16)
            nc.sync.dma_start(out=idx_sb[:], in_=idx[:])
            cof_sb = cpool.tile([128, 4 * T], f32)
            nc.sync.dma_start(out=cof_sb[:], in_=cof[:])
            idn_sb = cpool.tile([128, 128], bf16)
            nc.sync.dma_start(out=idn_sb[:], in_=idn[:])
            bia_sb = cpool.tile([128, 4], f32)
            nc.sync.dma_start(out=bia_sb[:], in_=bia[:])
            w1_sb = cpool.tile([128, 2 * C], bf16)
            nc.scalar.dma_start(out=w1_sb[:, 0:C], in_=w1[0:128, :])
            nc.scalar.dma_start(out=w1_sb[:, C : 2 * C], in_=w1[128:256, :])
            w2_sb = cpool.tile([128, 2 * C], bf16)
            nc.scalar.dma_start(out=w2_sb[:, 0:C], in_=w2[0:128, :])
            nc.scalar.dma_start(out=w2_sb[:, C : 2 * C], in_=w2[128:256, :])
            out_sb = cpool.tile([128, 2 * NP], bf16)

            # band gathers (Pool engine runs ONLY these; mlp library stays
            # resident)
            gts = []
            tbl_ap = tbl[:]
            band_cut = []
            acc = 0
            for kb in BANDS_T:
                band_cut.append((acc, acc + kb))
                acc += kb
            for b, kb in enumerate(BANDS_T):
                gt_t = gpool.tile([128, KBMAX, CB], bf16, name="gt")
                in_ap = bass.AP(
                    tbl_ap.tensor,
                    b * SLABROWS * CB,
                    [[CB, SLABROWS], [1, CB]],
                )
                num_idxs = kb * 128
                nc.gpsimd.dma_gather(
                    out_ap=gt_t[:, 0:kb, :],
                    in_ap=in_ap,
                    idxs_ap=idx_sb[:, idx_off[b] : idx_off[b + 1]],
                    num_idxs=num_idxs,
                    num_idxs_reg=num_idxs,
                    elem_size=CB,
                    elem_step=CB,
                    queue_num=b % 2,
                )
                gts.append(gt_t)

            def tile_src(t):
                for b, (t0, t1) in enumerate(band_cut):
                    if t < t1:
                        return gts[b], t - t0
                raise AssertionError(t)

            gelu_f = AF.Identity if os.environ.get("GRIDSTN_NOGELU") else AF.Gelu

            for pr in range(T // 2):
                px = pxp.tile([128, 512], f32, name="px")
                for tt in range(2):
                    t = pr * 2 + tt
                    gt_t, tl = tile_src(t)
                    dg = dpool.tile([128, 512], bf16, name="dg")
                    for j in range(4):
                        nc.vector.tensor_scalar_mul(
                            dg[:, j * 128 : (j + 1) * 128],
                            idn_sb[:],
                            cof_sb[:, j * T + t : j * T + t + 1],
                        )
                    for ch in range(2):
                        for j in range(4):
                            xo = j * 256 + ch * 128
                            nc.tensor.matmul(
                                out=px[:, ch * 256 + tt * 128 : ch * 256 + tt * 128 + 128],
                                lhsT=gt_t[:, tl, xo : xo + 128],
                                rhs=dg[:, j * 128 : (j + 1) * 128],
                                start=(j == 0),
                                stop=(j == 3),
                            )
                xs = xpool.tile([128, 512], bf16, name="xs")
                if pr % 2 == 1:
                    nc.scalar.activation(out=xs[:], in_=px[:], func=AF.Copy)
                else:
                    nc.vector.tensor_copy(xs[:], px[:])
                ph = php.tile([128, 512], f32, name="ph")
                for m in range(2):
                    for k in range(2):
                        nc.tensor.matmul(
                            out=ph[:, m * 256 : (m + 1) * 256],
                            lhsT=w1_sb[:, k * C + m * 128 : k * C + (m + 1) * 128],
                            rhs=xs[:, k * 256 : (k + 1) * 256],
                            start=(k == 0),
                            stop=(k == 1),
                        )
                hs = hpool.tile([128, 512], bf16, name="hs")
                if b1z:
                    nc.scalar.activation(out=hs[:], in_=ph[:], func=gelu_f)
                else:
                    for m in range(2):
                        nc.scalar.activation(
                            out=hs[:, m * 256 : (m + 1) * 256],
                            in_=ph[:, m * 256 : (m + 1) * 256],
                            func=gelu_f,
                            bias=bia_sb[:, m : m + 1],
                            scale=1.0,
                        )
                py = pyp.tile([128, 512], f32, name="py")
                for m in range(2):
                    for k in range(2):
                        nc.tensor.matmul(
                            out=py[:, m * 256 : (m + 1) * 256],
                            lhsT=w2_sb[:, k * C + m * 128 : k * C + (m + 1) * 128],
                            rhs=hs[:, k * 256 : (k + 1) * 256],
                            start=(k == 0),
                            stop=(k == 1),
                        )
                col = pr * 256
                yv = bass.AP(
                    out_sb[:].tensor,
                    out_sb[:].offset + col,
                    [out_sb[:].ap[0], [NP, 2], [1, 256]],
                )
                if b2z:
                    if pr % 2 == 1:
                        nc.scalar.activation(out=yv, in_=py[:], func=AF.Copy)
                    else:
                        nc.vector.tensor_copy(yv, py[:])
                else:
                    for m in range(2):
                        nc.scalar.activation(
                            out=out_sb[:, m * NP + col : m * NP + col + 256],
                            in_=py[:, m * 256 : (m + 1) * 256],
                            func=AF.Identity,
                            bias=bia_sb[:, 2 + m : 3 + m],
                            scale=1.0,
                        )
                if pr % 2 == 1:
                    c0 = (pr - 1) * 256
                    for m in range(2):
                        nc.sync.dma_start(
                            out=out[m, :, c0 : c0 + 512],
                            in_=out_sb[:, m * NP + c0 : m * NP + c0 + 512],
                        )
    return nc


def _pack_idx(idx_flat):
    """idx list (len = n*256, order: position j = blk*128+p) -> int16 SBUF
    layout [128, n*16]: entry j lives at (partition j%16, col j//16),
    replicated across the 8 groups of 16 partitions."""
    n = len(idx_flat) // 16
    arr = np.zeros((128, n), np.int16)
    block = np.asarray(idx_flat, np.int16).reshape(n, 16).T  # [16, n]
    for g in range(8):
        arr[g * 16 : (g + 1) * 16, :] = block
    return arr


def _make_in_maps(grid_features, station_coords, W1, b1, W2, b2):
    import jax
    import jax.numpy as jnp

    ix0, iy0, iy1, cjs = _host_route(station_coords)

    order0 = np.argsort(ix0, kind="stable")
    chunks = []
    los = []
    for c in range(NCORES):
        ch = order0[c * NPC : (c + 1) * NPC]
        ch = ch[np.argsort(iy0[ch], kind="stable")]
        chunks.append(ch)
        a = int(ix0[ch].min())
        b = int(ix0[ch].max())
        assert b - a + 2 <= WTBL, f"core {c} column spread {b - a} exceeds WTBL"
        los.append(a)

    # band cut positions in tiles
    band_t0 = np.cumsum([0] + BANDS_T).tolist()

    with jax.default_device(jax.devices("cpu")[0]):
        g = jnp.asarray(np.asarray(grid_features[0]))  # (C,H,W) f32
        gt = np.asarray(jnp.transpose(g, (1, 2, 0)).astype(jnp.bfloat16))  # (H,W,C)
        w1t = np.ascontiguousarray(
            np.asarray(jnp.asarray(np.asarray(W1, np.float32).T).astype(jnp.bfloat16))
        )
        w2t = np.ascontiguousarray(
            np.asarray(jnp.asarray(np.asarray(W2, np.float32).T).astype(jnp.bfloat16))
        )
        idn = np.asarray(
            jnp.asarray(np.eye(128, dtype=np.float32)).astype(jnp.bfloat16)
        )
    bia = np.zeros((128, 4), np.float32)
    bia[:, 0] = b1[0:128]
    bia[:, 1] = b1[128:256]
    bia[:, 2] = b2[0:128]
    bia[:, 3] = b2[128:256]

    in_maps = []
    for c in range(NCORES):
        sids = chunks[c]
        cols = np.clip(np.arange(los[c], los[c] + WTBL), 0, W - 1)
        gtc = np.ascontiguousarray(gt[:, cols, :])  # (H, WTBL, C) bf16
        # 4-corner blocks: blk[y, x] = [g[y,x], g[y,x+1], g[y+1,x], g[y+1,x+1]]
        gp = np.concatenate([gtc, gtc[:, -1:, :]], axis=1)
        gp = np.concatenate([gp, gp[-1:, :, :]], axis=0)  # (H+1, WTBL+1, C)
        blk = np.concatenate(
            [gp[:-1, :-1], gp[:-1, 1:], gp[1:, :-1], gp[1:, 1:]], axis=2
        )  # (H, WTBL, 4C)
        x0l = (ix0[sids] - los[c]).astype(np.int64)
        tblc = np.zeros((TROWS, CB), gt.dtype)
        idx_parts = []
        for bnd in range(NBANDS):
            t0, t1 = band_t0[bnd], band_t0[bnd + 1]
            s0, s1 = t0 * 128, t1 * 128
            bids = np.arange(s0, s1)
            ybase = int(iy0[sids[s0]])
            yend = int(iy0[sids[s1 - 1]])
            nrow = (yend - ybase + 1) * WTBL
            assert nrow <= SLABROWS, f"band {bnd} rows {nrow} > {SLABROWS}"
            tblc[bnd * SLABROWS : bnd * SLABROWS + nrow] = blk[
                ybase : yend + 1
            ].reshape(nrow, CB)
            r0 = (iy0[sids[bids]] - ybase).astype(np.int64) * WTBL + x0l[bids]
            assert r0.max() <= MAX_IDX
            idx_parts.append(_pack_idx(r0))
        idx_arr = np.concatenate(idx_parts, axis=1)
        cof_arr = np.ascontiguousarray(
            np.concatenate(
                [cjs[j][sids].astype(np.float32).reshape(T, 128).T for j in range(4)],
                axis=1,
            )
        )
        in_maps.append(
            {
                "tbl": tblc,
                "idx": np.ascontiguousarray(idx_arr),
                "cof": cof_arr,
                "w1t": w1t,
                "w2t": w2t,
                "bia": bia,
                "idn": idn,
            }
        )
    return in_maps, chunks


def _install_ntff_shim():
    import sys
    import types

    try:
        import antenv.axon_hooks  # noqa: F401

        return
    except ImportError:
        pass
    from trn_agent_boot.trn_boot import _ntff_profile_via_ctypes

    hook = _ntff_profile_via_ctypes("/opt/axon/libaxon_pjrt.so")
    mod = types.ModuleType("antenv.axon_hooks")
    mod.get_axon_ntff_profile_hook = lambda: hook
    mod.set_axon_ntff_profile_hook = lambda h: None
    sys.modules["antenv.axon_hooks"] = mod


def _get_program(b1z=False, b2z=False):
    key = (b1z, b2z, bool(os.environ.get("GRIDSTN_NOGELU")))
    if key not in _PROG_CACHE:
        _PROG_CACHE[key] = _build_program(b1z, b2z)
    return _PROG_CACHE[key]


def kernel(grid_features, station_coords, W1, b1, W2, b2):
    in_maps, chunks = _make_in_maps(
        grid_features, station_coords, W1, b1, W2, b2
    )
    b1z = not np.any(np.asarray(b1))
    b2z = not np.any(np.asarray(b2))
    nc = _get_program(b1z, b2z)

    from concourse.bass_utils import run_bass_kernel_spmd

    trace = bool(os.environ.get("GRIDSTN_TRACE"))
    if trace:
        _install_ntff_shim()
    if not nc.is_finalized():
        nc.finalize()
    res = run_bass_kernel_spmd(nc, in_maps, list(range(NCORES)), trace=trace)
    LAST_RUN_INFO["exec_time_ns"] = res.exec_time_ns
    LAST_RUN_INFO["mean_exec_time_ns"] = res.mean_exec_time_ns
    LAST_RUN_INFO["profile_json"] = res.profile_json
    outs = [np.asarray(r["out"], np.float32) for r in res.results]

    result = np.zeros((N, C), np.float32)
    for c in range(NCORES):
        y = outs[c].reshape(2 * 128, NP)
        result[chunks[c]] = y.T
    return result.reshape(B, N, C)
